# revision 39
# baseline (speedup 1.0000x reference)
"""BinaryMemoryRNNCell Trainium kernel v7.

Batch-sharded over 8 cores (16 rows/core), no collectives. Weights and
activations single bf16 (host-verified rel err ~7.5e-3 vs the 2e-2 gate);
the index path stays f32 (min |logit| = 2.4e-5). Weight stream is 8MB/core
across THREE HWDGE queues (sync/scalar/vector) in 16 column-chunk units so
the 4 PSUM chunks close progressively. LayerNorm stats (sum/sumsq) are
computed per chunk on DVE as chunks close; the tail is a short scalar chain
(bit-magic Newton rsqrt) fused into one sigmoid ACT via scale/bias, then a
single 64KB output DMA. mem is staged bf16 (half the gather bytes).
"""
import numpy as np
import ml_dtypes
import concourse.bass as bass
from concourse import mybir
from concourse.bass import IndirectOffsetOnAxis
from concourse.bass_utils import run_bass_kernel_spmd


# ---------------------------------------------------------------------------
# Tile workarounds for this container's walrus build (max ONE sync wait per
# instruction): split the exit-drain's waits across single-wait NOPs, and a
# post-lowering pass that does the same for every multi-wait instruction.
# ---------------------------------------------------------------------------
import concourse.tile as _tile
from concourse.vector_clock import ScopedClock, VectorClock
from concourse.tile_sem_assignment import N_PROCS


class TileContextSplitDrain(_tile.TileContext):
    def _drain_and_barrier(self, tick_clock, wait_clock):
        gc = tick_clock.global_clock
        vals = [gc[p] for p in range(N_PROCS)]
        for base in range(N_PROCS):
            chunk_vals = [vals[p] if p == base else 0 for p in range(N_PROCS)]
            if not any(chunk_vals):
                continue
            nop_inst = self.nc.sync.nop(nofuse=True)
            wait_clock.add_sem_waits(
                nop_inst.ins, ScopedClock({None: VectorClock(chunk_vals)})
            )
        # The NOPs above (same engine, program order) already waited on the
        # full global clock; the drain needs no waits of its own.
        self.nc.sync.drain()
        self.nc.all_engine_barrier()
        assert self.sems is not None
        popped = self.nc._tile_sem_poison_stack.pop()
        assert popped is self._sem_poison
        self.nc.clear_and_free_semaphores(list(self.sems.allocated().values()))
        self.nc.all_engine_barrier()


def split_multi_waits(nc, max_waits=1):
    counter = 0
    for func in nc.m.functions:
        for bb in func.blocks:
            out = []
            changed = False
            for inst in bb.instructions:
                si = inst.sync_info
                if si is not None and len(si.on_wait) > max_waits:
                    waits = list(si.on_wait)
                    for w in waits[:-max_waits]:
                        counter += 1
                        out.append(mybir.InstNoOp(
                            name=f"waitsplit_{counter}",
                            engine=inst.engine,
                            bass_nofuse=True,
                            sync_info=mybir.SyncInfo(on_wait=[w], on_update=[]),
                        ))
                    inst.sync_info = mybir.SyncInfo(
                        on_wait=waits[-max_waits:], on_update=list(si.on_update))
                    changed = True
                out.append(inst)
            if changed:
                bb.instructions = out
    return counter

F32 = mybir.dt.float32
BF16 = mybir.dt.bfloat16
I32 = mybir.dt.int32
AO = mybir.AluOpType

NC = 8
B = 128
BL = 16
H = 1024
T = 1024
NB = 10
LN_EPS = 1e-5
NCH = 4          # output column chunks
CW = H // NCH    # 256 cols per chunk

# Per-queue unit order (matrix, chunk), 512KB units. cfc rides first on sync
# (HWDGE starts fast; SWDGE needs ~12.5us to emit its first packet), cbc/bpr
# on scalar. gpsimd does the gathers then 4 weight units. PE consumes in
# estimated arrival order; the stream's last two units both belong to c2, so
# only c2's evac+stats land in the tail (c3 is fully delivered by ~31us).
Q_SYNC = [(0, 0), (0, 1), (2, 0), (2, 1), (0, 3), (0, 2)]
Q_SCAL = [(1, 0), (1, 1), (3, 0), (3, 1), (1, 3), (3, 2)]
Q_GPS = [(1, 2), (2, 2), (2, 3), (3, 3)]
PE_EARLY = [(0, 0), (1, 0)]
PE_MID = [(1, 2), (0, 1), (1, 1)]
PE_LATE1 = [(2, 2), (2, 0), (3, 0), (2, 3), (2, 1), (3, 1),
            (0, 3), (1, 3), (3, 3)]
PE_LATE2 = [(0, 2), (3, 2)]
CLOSE_UNIT = {0: (3, 0), 1: (3, 1), 2: (3, 2), 3: (3, 3)}
N_FILL = 20  # PE-warming junk matmuls bridging the final arrival gap

_CACHED = {}


def _chunked_T(a):
    K, M = a.shape
    out = a.reshape(K // 128, 128, M).transpose(1, 0, 2).reshape(128, (K // 128) * M)
    return np.ascontiguousarray(out)


def _wlayout(Wt):
    """[1024 in, 1024 out] -> [128, NCH*8*CW] chunk-major (c, k, j)."""
    v = Wt.reshape(8, 128, NCH, CW).transpose(1, 2, 0, 3)
    return np.ascontiguousarray(v.reshape(128, NCH * 8 * CW))


def build(split=True, ln_trivial=False):
    nc = bass.Bass()
    p = {}
    p["mem"] = nc.declare_dram_parameter("mem", [T * BL, H], BF16, isOutput=False)
    for m in range(4):
        p[f"wh{m}"] = nc.declare_dram_parameter(f"wh{m}", [128, 8192], BF16, isOutput=False)
    # critical consts: hT | mwt | id16/powmat/iota/mbrow/ones1 (f32)
    p["cfc"] = nc.declare_dram_parameter("cfc", [128, 344], F32, isOutput=False)
    # xb | hb (bf16 single) + onesb + id16b
    p["cbc"] = nc.declare_dram_parameter("cbc", [128, 288], BF16, isOutput=False)
    p["bpair"] = nc.declare_dram_parameter("bpair", [8, H], BF16, isOutput=False)
    if not ln_trivial:
        p["lnr"] = nc.declare_dram_parameter("lnr", [BL, 2 * H], F32, isOutput=False)
    y = nc.declare_dram_parameter("y", [BL, H], F32, isOutput=True)

    with TileContextSplitDrain(nc) as tc:
        with (
            tc.tile_pool(name="const", bufs=1) as cpool,
            tc.tile_pool(name="work", bufs=1) as wk,
            tc.tile_pool(name="wts", bufs=1) as wpool,
            tc.tile_pool(name="psum_small", bufs=3, space="PSUM") as psmall,
            tc.tile_pool(name="psum_main", bufs=1, space="PSUM") as pmain,
        ):
            # ---- consts: critical cfc (hT) on sync, small ones on scalar ----
            cfc = cpool.tile([128, 344], F32, name="cfc")
            cbc = cpool.tile([128, 288], BF16, name="cbc")
            bpr = cpool.tile([8, H], BF16, name="bpr")
            nc.sync.dma_start(cfc[:], p["cfc"][:])
            nc.scalar.dma_start(cbc[:], p["cbc"][:])
            nc.scalar.dma_start(bpr[:], p["bpair"][:])
            sb = {
                "hT": cfc[:, 0:128],
                "mwt": cfc[:, 128:288],
                "id16": cfc[0:16, 288:304],
                "powmat": cfc[0:20, 304:306],
                "iota": cfc[0:16, 306:307],
                "mbrow": cfc[0:1, 307:327],
                "ones1": cfc[0:1, 327:343],
                "xb": cbc[:, 0:128],
                "hb": cbc[:, 128:256],
                "onesb": cbc[0:8, 256:272],
                "id16b": cbc[0:16, 272:288],
            }

            wtiles = [wpool.tile([128, 8192], BF16, name=f"wh{m}") for m in range(4)]

            def wdma(eng, m, c):
                cs = slice(c * 8 * CW, (c + 1) * 8 * CW)
                eng.dma_start(wtiles[m][:, cs], p[f"wh{m}"][:, cs])

            # This build's HWDGE queues are sync (SP) + scalar (Activation)
            # only; four units ride the gpsimd SWDGE queue behind the gathers.
            for m, c in Q_SYNC:
                wdma(nc.sync, m, c)
            for m, c in Q_SCAL:
                wdma(nc.scalar, m, c)

            # ---- ACT table pre-warm (sigmoid) ----
            warm = wk.tile([1, 1], F32)
            nc.vector.memset(warm[:], 0.25)
            warm2 = wk.tile([1, 1], F32)
            nc.scalar.activation(warm2[:], warm[:], mybir.ActivationFunctionType.Sigmoid)

            # ---- logits -> bits -> flat gather indices (f32 exactness) ----
            ps_lg = psmall.tile([BL, 2 * NB], F32, tag="small")
            nc.tensor.matmul(ps_lg[:], lhsT=sb["ones1"],
                             rhs=sb["mbrow"], start=True, stop=False)
            for k in range(8):
                nc.tensor.matmul(
                    ps_lg[:],
                    lhsT=sb["hT"][:, k * BL:(k + 1) * BL],
                    rhs=sb["mwt"][:, k * 2 * NB:(k + 1) * 2 * NB],
                    start=False, stop=(k == 7),
                )
            bits = wk.tile([BL, 2 * NB], F32)
            nc.vector.tensor_scalar(bits[:], ps_lg[:], 0.0, None, AO.is_gt)
            ps_bt = psmall.tile([2 * NB, BL], F32, tag="small")
            nc.tensor.transpose(ps_bt[:], bits[:], sb["id16"])
            bitsT = wk.tile([2 * NB, BL], F32)
            nc.vector.tensor_copy(bitsT[:], ps_bt[:])
            ps_idx = psmall.tile([BL, 2], F32, tag="small")
            nc.tensor.matmul(ps_idx[:], lhsT=bitsT[:], rhs=sb["powmat"],
                             start=True, stop=True)
            flatf = wk.tile([BL, 2], F32)
            nc.vector.tensor_scalar(flatf[:], ps_idx[:], float(BL), sb["iota"],
                                    AO.mult, AO.add)
            flati = wk.tile([BL, 2], I32)
            nc.vector.tensor_copy(flati[:], flatf[:])

            # ---- gathers (SWDGE), mem staged bf16 ----
            hr = wk.tile([BL, H], BF16)
            hl = wk.tile([BL, H], BF16)
            nc.gpsimd.indirect_dma_start(
                out=hr[:], out_offset=None, in_=p["mem"][:],
                in_offset=IndirectOffsetOnAxis(ap=flati[:, 0:1], axis=0))
            nc.gpsimd.indirect_dma_start(
                out=hl[:], out_offset=None, in_=p["mem"][:],
                in_offset=IndirectOffsetOnAxis(ap=flati[:, 1:2], axis=0))

            # weight units on the SWDGE queue, behind the gathers
            for m, c in Q_GPS:
                wdma(nc.gpsimd, m, c)

            # ---- bias into the 4 chunk PSUMs ----
            psC = [pmain.tile([BL, CW], F32, tag=f"C{c}", name=f"psC{c}")
                   for c in range(NCH)]
            for c in range(NCH):
                nc.tensor.matmul(psC[c][:], lhsT=sb["onesb"],
                                 rhs=bpr[:, c * CW:(c + 1) * CW],
                                 start=True, stop=False)

            # tail tiles
            pre_sb = wk.tile([BL, H], F32)
            sums = wk.tile([BL, NCH], F32)
            sumsqs = wk.tile([BL, NCH], F32)
            ysb = wk.tile([BL, H], F32)

            # memp[:, j*128 + k*16 : ...] = bf16 transposed gather chunks
            memp = wk.tile([128, 256], BF16)

            def mains(units):
                for m, c in units:
                    if m == 0:
                        lp = sb["xb"]
                    elif m == 1:
                        lp = sb["hb"]
                    else:
                        lp = memp[:, (m - 2) * 128:(m - 1) * 128]
                    closing = CLOSE_UNIT[c] == (m, c)
                    for k in range(8):
                        rs = slice(c * 8 * CW + k * CW, c * 8 * CW + (k + 1) * CW)
                        nc.tensor.matmul(
                            psC[c][:], lhsT=lp[:, k * BL:(k + 1) * BL],
                            rhs=wtiles[m][:, rs],
                            start=False, stop=(closing and k == 7))
                    if closing:
                        # chunk closed: evacuate + partial LN stats
                        cs = slice(c * CW, (c + 1) * CW)
                        nc.vector.tensor_copy(pre_sb[:, cs], psC[c][:])
                        nc.vector.reduce_sum(sums[:, c:c + 1], pre_sb[:, cs],
                                             axis=mybir.AxisListType.X)
                        sqt = wk.tile([BL, CW], F32, name="sq", tag="sq", bufs=2)
                        nc.gpsimd.tensor_tensor(sqt[:], pre_sb[:, cs],
                                                pre_sb[:, cs], AO.mult)
                        nc.vector.reduce_sum(sumsqs[:, c:c + 1], sqt[:],
                                             axis=mybir.AxisListType.X)

            # W/U of c0,c1 can run before the gather lands
            mains(PE_EARLY)

            # ---- transpose gathered rows into bf16 lhsT layout ----
            for j, src in enumerate((hr, hl)):
                for k in range(8):
                    ps_t = psmall.tile([128, BL], BF16, tag="small", name=f"ps_t{j}_{k}")
                    nc.tensor.transpose(ps_t[:], src[:, k * 128:(k + 1) * 128],
                                        sb["id16b"])
                    nc.vector.tensor_copy(memp[:, j * 128 + k * BL:
                                               j * 128 + (k + 1) * BL], ps_t[:])

            mains(PE_MID)
            mains(PE_LATE1)

            # ---- speculative rsqrt on the 768 cols of c0/c1/c3 (done ~31us):
            # seed + one Newton iteration here; one refinement in the tail.
            sp = wk.tile([BL, 1], F32)
            qp = wk.tile([BL, 1], F32)
            nc.vector.tensor_tensor(sp[:], sums[:, 0:1], sums[:, 1:2], AO.add)
            nc.vector.tensor_tensor(sp[:], sp[:], sums[:, 3:4], AO.add)
            nc.vector.tensor_tensor(qp[:], sumsqs[:, 0:1], sumsqs[:, 1:2], AO.add)
            nc.vector.tensor_tensor(qp[:], qp[:], sumsqs[:, 3:4], AO.add)
            Hp = float(3 * CW)
            munp = wk.tile([BL, 1], F32)
            nc.vector.tensor_scalar(munp[:], sp[:], -1.0 / Hp, None, AO.mult)
            mu2p = wk.tile([BL, 1], F32)
            nc.vector.tensor_scalar(mu2p[:], munp[:], munp[:, 0:1], -LN_EPS,
                                    AO.mult, AO.add)
            vep = wk.tile([BL, 1], F32)
            nc.vector.tensor_scalar(vep[:], qp[:], 1.0 / Hp, mu2p[:, 0:1],
                                    AO.mult, AO.subtract)
            yv = wk.tile([BL, 1], F32)
            nc.vector.tensor_scalar(yv[:].bitcast(I32), vep[:].bitcast(I32), 1,
                                    None, AO.logical_shift_right)
            nc.vector.tensor_scalar(yv[:].bitcast(I32), yv[:].bitcast(I32), -1,
                                    0x5F3759DF, AO.mult, AO.add)
            t1 = wk.tile([BL, 1], F32)
            nc.vector.tensor_scalar(t1[:], yv[:], yv[:, 0:1], vep[:, 0:1],
                                    AO.mult, AO.mult)
            nc.vector.tensor_scalar(t1[:], t1[:], -0.5, 1.5, AO.mult, AO.add)
            nc.vector.tensor_scalar(yv[:], yv[:], t1[:, 0:1], None, AO.mult)

            # junk matmuls keep the PE p-state ramped until the final units land
            psJ = pmain.tile([BL, CW], F32, tag="junk", name="psJ")
            for _ in range(N_FILL):
                nc.tensor.matmul(psJ[:], lhsT=sb["xb"][:, 0:BL],
                                 rhs=cbc[:, 0:CW], start=True, stop=True)

            mains(PE_LATE2)

            # ---- LayerNorm scalars (final: full 1024-col stats) ----
            S = wk.tile([BL, 1], F32)
            Q = wk.tile([BL, 1], F32)
            nc.vector.reduce_sum(S[:], sums[:], axis=mybir.AxisListType.X)
            nc.vector.reduce_sum(Q[:], sumsqs[:], axis=mybir.AxisListType.X)
            mun = wk.tile([BL, 1], F32)   # -mu
            nc.vector.tensor_scalar(mun[:], S[:], -1.0 / H, None, AO.mult)
            mu2 = wk.tile([BL, 1], F32)   # mu^2 - eps
            nc.vector.tensor_scalar(mu2[:], mun[:], mun[:, 0:1], -LN_EPS,
                                    AO.mult, AO.add)
            ve = wk.tile([BL, 1], F32)    # q/H + eps - mu^2
            nc.vector.tensor_scalar(ve[:], Q[:], 1.0 / H, mu2[:, 0:1],
                                    AO.mult, AO.subtract)
            nc.vector.tensor_scalar(t1[:], yv[:], yv[:, 0:1], ve[:, 0:1],
                                    AO.mult, AO.mult)
            nc.vector.tensor_scalar(t1[:], t1[:], -0.5, 1.5, AO.mult, AO.add)
            nc.vector.tensor_scalar(yv[:], yv[:], t1[:, 0:1], None, AO.mult)
            nb_ = wk.tile([BL, 1], F32)   # -mu * inv
            nc.vector.tensor_scalar(nb_[:], mun[:], yv[:, 0:1], None, AO.mult)

            # ---- fused normalize + sigmoid, then the output DMA ----
            if ln_trivial:
                nc.scalar.activation(ysb[:], pre_sb[:],
                                     mybir.ActivationFunctionType.Sigmoid,
                                     bias=nb_[:, 0:1], scale=yv[:, 0:1])
            else:
                lnr = cpool.tile([BL, 2 * H], F32, name="lnr")
                nc.sync.dma_start(lnr[:], p["lnr"][:])
                normed = wk.tile([BL, H], F32)
                nc.vector.tensor_scalar(normed[:], pre_sb[:], yv[:, 0:1],
                                        nb_[:, 0:1], AO.mult, AO.add)
                nc.vector.tensor_tensor(normed[:], normed[:], lnr[:, 0:H], AO.mult)
                nc.vector.tensor_tensor(normed[:], normed[:], lnr[:, H:2 * H], AO.add)
                nc.scalar.activation(ysb[:], normed[:],
                                     mybir.ActivationFunctionType.Sigmoid)
            nc.sync.dma_start(y[:, 0:H // 2], ysb[:, 0:H // 2])
            nc.scalar.dma_start(y[:, H // 2:H], ysb[:, H // 2:H])
    if split:
        split_multi_waits(nc)
    return nc


def _prep_host(x, h_prev, mem_tensor, W_w, W_b, U_w, U_b, M_w, M_b,
               Qr_w, Qr_b, Ql_w, Ql_b, ln_g, ln_b):
    shared = {}
    mwt = _chunked_T(np.ascontiguousarray(M_w.T))
    for m, W in enumerate((W_w, U_w, Qr_w, Ql_w)):
        shared[f"wh{m}"] = _wlayout(
            np.ascontiguousarray(W.T)).astype(ml_dtypes.bfloat16)
    bst = np.stack([W_b, U_b, Qr_b, Ql_b])
    bh = bst.astype(ml_dtypes.bfloat16)
    bl = (bst - bh.astype(np.float32)).astype(ml_dtypes.bfloat16)
    shared["bpair"] = np.ascontiguousarray(np.concatenate([bh, bl], axis=0))

    powmat = np.zeros((2 * NB, 2), np.float32)
    powmat[:NB, 0] = 2.0 ** np.arange(NB - 1, -1, -1)
    powmat[NB:, 1] = 2.0 ** np.arange(NB - 1, -1, -1)
    cfc = np.zeros((128, 344), np.float32)
    cfc[:, 128:288] = mwt
    cfc[0:16, 288:304] = np.eye(16)
    cfc[0:20, 304:306] = powmat
    cfc[0:16, 306:307] = np.arange(BL)[:, None]
    cfc[0:1, 307:327] = M_b[None, :]
    cfc[0:1, 327:343] = 1.0
    shared["cfc_base"] = cfc

    ln_trivial = bool(np.all(ln_g == 1.0) and np.all(ln_b == 0.0))
    if not ln_trivial:
        lnr = np.zeros((BL, 2 * H), np.float32)
        lnr[:, 0:H] = ln_g[None, :]
        lnr[:, H:2 * H] = ln_b[None, :]
        shared["lnr"] = lnr

    mem_b = mem_tensor.astype(ml_dtypes.bfloat16)

    per_core = []
    for c in range(NC):
        bs = slice(c * BL, (c + 1) * BL)
        d = {k: v for k, v in shared.items() if k != "cfc_base"}
        xt = _chunked_T(np.ascontiguousarray(x[bs].T))
        ht = _chunked_T(np.ascontiguousarray(h_prev[bs].T))
        cbc = np.zeros((128, 288), ml_dtypes.bfloat16)
        cbc[:, 0:128] = xt.astype(ml_dtypes.bfloat16)
        cbc[:, 128:256] = ht.astype(ml_dtypes.bfloat16)
        cbc[0:8, 256:272] = 1.0
        cbc[0:16, 272:288] = np.eye(16)
        d["cbc"] = np.ascontiguousarray(cbc)
        cfc_c = shared["cfc_base"].copy()
        cfc_c[:, 0:128] = ht
        d["cfc"] = cfc_c
        d["mem"] = np.ascontiguousarray(mem_b[:, bs, :]).reshape(T * BL, H)
        per_core.append(d)
    return per_core, ln_trivial


def kernel(**inputs):
    in_maps, ln_trivial = _prep_host(**{k: np.asarray(v) for k, v in inputs.items()})
    key = ("nc", ln_trivial)
    if key not in _CACHED:
        _CACHED[key] = build(ln_trivial=ln_trivial)
    nc = _CACHED[key]
    res = run_bass_kernel_spmd(nc, in_maps, list(range(NC)),
                               trace=bool(_CACHED.get("trace")))
    _CACHED["last_result"] = res
    return np.concatenate([res.results[c]["y"] for c in range(NC)], axis=0)
